# revision 9
# baseline (speedup 1.0000x reference)
"""Expert-parallel MoE MLP kernel for Trainium2 (8 NeuronCores, 1 expert/core).

Problem: inputs [1, 8, 16384, 512], per-expert 2-layer GELU MLP
  h   = gelu(x @ W1[e] + b1[e])      # [16384, 2048]
  out = h @ W2[e] + b2[e]            # [16384, 512]

x is transposed host-side (numpy) so the kernel receives xT [D, C] and
needs NO on-chip transposes; d (layer-1 contraction) is already on
partitions.

Per-core dataflow:
  1. DMA xT block [d=512, t=512] -> SBUF [128p, kd, t] (prefetch ahead)
  2. L1: psum[f,t] = sum_k matmul(lhsT=W1[dk, f], rhs=xT[dk, t])   (fp32r)
  3. ScalarE Gelu(+b1 per-partition bias) psum -> hT sbuf [f, t]
  4. L2: psum[t,d'] = sum_k matmul(lhsT=hT[fk, t], rhs=W2[fk, d']) (fp32r)
     -> output lands in natural token-major layout
  5. DVE add b2 (broadcast) psum -> sbuf, DMA out.
"""

import os
import numpy as np

E, C, D, F = 8, 16384, 512, 2048
P = 128
TBLK = 512  # tokens per block
MM_DT = "bfloat16"  # moving operand may stream 2 cols/cyc on HW (vs fp32r 1)

_CACHE = {}


def _build(T, act="Gelu_apprx_tanh"):
    import concourse.mybir as mybir
    import concourse.tile as tile
    from concourse import bacc

    f32 = mybir.dt.float32
    mm_dt = getattr(mybir.dt, MM_DT)
    gelu_fn = getattr(mybir.ActivationFunctionType, act)

    nc = bacc.Bacc("TRN2", target_bir_lowering=False, debug=False)

    xt_d = nc.dram_tensor("xt", [D, T], mm_dt, kind="ExternalInput").ap()
    w1_d = nc.dram_tensor("w1", [D, F], mm_dt, kind="ExternalInput").ap()
    b1_d = nc.dram_tensor("b1", [F], f32, kind="ExternalInput").ap()
    w2_d = nc.dram_tensor("w2", [F, D], mm_dt, kind="ExternalInput").ap()
    b2_d = nc.dram_tensor("b2", [D], f32, kind="ExternalInput").ap()
    o_d = nc.dram_tensor("out", [T, D], f32, kind="ExternalOutput").ap()

    KD = D // P   # 4  k-tiles (d) for layer 1
    KF = F // P   # 16 k-tiles (f) for layer 2
    NB = T // TBLK
    JT = TBLK // P  # 4 token sub-tiles per block

    xt_r = xt_d.rearrange("(k p) t -> p k t", p=P)

    with tile.TileContext(nc) as tc:
        with (
            tc.tile_pool(name="consts", bufs=1) as consts,
            tc.tile_pool(name="xt", bufs=3) as xt_pool,
            tc.tile_pool(name="ht", bufs=2) as ht_pool,
            tc.tile_pool(name="ot", bufs=3) as ot_pool,
            tc.tile_pool(name="ph", bufs=3, space="PSUM") as ph_pool,
            tc.tile_pool(name="po", bufs=4, space="PSUM") as po_pool,
            tc.tile_pool(name="warm", bufs=1, space="PSUM") as warm_pool,
        ):
            def load_block(blk):
                xt = xt_pool.tile([P, KD, TBLK], mm_dt, name="xt", tag="xt")
                t0 = blk * TBLK
                for k in range(KD):
                    nc.sync.dma_start(xt[:, k, :], xt_r[:, k, t0 : t0 + TBLK])
                return xt

            # --- setup ---
            # PE warmup: dummy matmuls on a zeroed scratch tile run during the
            # initial DMA wait so the HAM clock-gate opens (1.2 -> 2.4 GHz)
            # before the first real matmul.
            scratch = consts.tile([P, TBLK], mm_dt)
            nc.gpsimd.memset(scratch[:], 0)
            ph_w = warm_pool.tile([P, TBLK], f32)
            for _ in range(18):
                nc.tensor.matmul(
                    ph_w[:], scratch[:, :P], scratch[:], start=True, stop=True
                )

            # DMA order tracks the startup critical path: biases first (gelu
            # f0 gates ph psum recycling), then xt0/w1 interleaved in the
            # order layer 1 consumes them.
            b1_sb = consts.tile([P, KF], f32)
            nc.sync.dma_start(b1_sb[:], b1_d.rearrange("(k p) -> p k", p=P))
            b2_bc = consts.tile([P, D], f32)
            nc.sync.dma_start(b2_bc[:], b2_d.unsqueeze(0).partition_broadcast(P))

            w1_sb = consts.tile([P, KD, F], mm_dt)
            w1_r = w1_d.rearrange("(k p) f -> p k f", p=P)
            FB = F // 4  # 512-wide f chunks, consumption order
            xts = {}
            xt0 = xt_pool.tile([P, KD, TBLK], mm_dt, name="xt", tag="xt")
            for k in range(KD):
                nc.sync.dma_start(xt0[:, k, :], xt_r[:, k, 0:TBLK])
                nc.sync.dma_start(
                    w1_sb[:, k, 0:FB], w1_r[:, k, 0:FB]
                )
            xts[0] = xt0
            for fb in range(1, 4):
                for k in range(KD):
                    nc.sync.dma_start(
                        w1_sb[:, k, fb * FB : (fb + 1) * FB],
                        w1_r[:, k, fb * FB : (fb + 1) * FB],
                    )

            xts[1] = load_block(1)

            w2_sb = consts.tile([P, KF, D], mm_dt)
            w2_r = w2_d.rearrange("(k p) d -> p k d", p=P)
            for k in range(KF):
                nc.sync.dma_start(w2_sb[:, k, :], w2_r[:, k, :])

            def layer1(xt):
                hts = []
                for f in range(KF):
                    ph = ph_pool.tile([P, TBLK], f32)
                    for k in range(KD):
                        nc.tensor.matmul(
                            ph[:],
                            w1_sb[:, k, f * P : (f + 1) * P],
                            xt[:, k, :],
                            start=(k == 0),
                            stop=(k == KD - 1),
                        )
                    ht_f = ht_pool.tile(
                        [P, TBLK], mm_dt, name=f"ht{f}", tag=f"ht{f}"
                    )
                    nc.scalar.activation(
                        ht_f[:], ph[:], gelu_fn, bias=b1_sb[:, f : f + 1]
                    )
                    hts.append(ht_f)
                return hts

            def layer2(blk, hts):
                t0 = blk * TBLK
                for j in range(JT):
                    po = po_pool.tile([P, D], f32)
                    for k in range(KF):
                        nc.tensor.matmul(
                            po[:],
                            hts[k][:, j * P : (j + 1) * P],
                            w2_sb[:, k, :],
                            start=(k == 0),
                            stop=(k == KF - 1),
                        )
                    ot_j = ot_pool.tile([P, D], f32, name=f"ot{j}", tag=f"ot{j}")
                    nc.vector.tensor_add(ot_j[:], po[:], b2_bc[:])
                    nc.sync.dma_start(
                        o_d[t0 + j * P : t0 + (j + 1) * P, :], ot_j[:]
                    )

            for blk in range(NB):
                if blk + 2 < NB:
                    xts[blk + 2] = load_block(blk + 2)
                hts = layer1(xts.pop(blk))
                layer2(blk, hts)

    nc.compile()
    return nc


def _get_nc(T):
    if T not in _CACHE:
        _CACHE[T] = _build(T)
    return _CACHE[T]


def kernel(inputs, W1, b1, W2, b2):
    import ml_dtypes
    from concourse.bass_utils import run_bass_kernel_spmd

    bf16 = ml_dtypes.bfloat16
    inputs = np.asarray(inputs, dtype=np.float32)
    W1 = np.ascontiguousarray(np.asarray(W1, dtype=np.float32).astype(bf16))
    b1 = np.ascontiguousarray(np.asarray(b1, dtype=np.float32))
    W2 = np.ascontiguousarray(np.asarray(W2, dtype=np.float32).astype(bf16))
    b2 = np.ascontiguousarray(np.asarray(b2, dtype=np.float32))
    # host-side transpose: feed xT [D, C] per expert (d on partitions)
    xT = np.ascontiguousarray(inputs[0].transpose(0, 2, 1).astype(bf16))

    nc = _get_nc(C)
    in_maps = [
        {
            "xt": xT[e],
            "w1": W1[e],
            "b1": b1[e],
            "w2": W2[e],
            "b2": b2[e],
        }
        for e in range(E)
    ]
    trace = os.environ.get("KERNEL_TRACE", "0") == "1"
    res = run_bass_kernel_spmd(
        nc, in_maps, core_ids=list(range(E)), trace=trace
    )
    if trace:
        kernel.last_exec_time_ns = res.exec_time_ns
    out = np.stack([res.results[e]["out"] for e in range(E)], axis=0)[None]
    return out


# revision 11
# speedup vs baseline: 1.0047x; 1.0047x over previous
"""Expert-parallel MoE MLP kernel for Trainium2 (8 NeuronCores, 1 expert/core).

Problem: inputs [1, 8, 16384, 512], per-expert 2-layer GELU MLP
  h   = gelu(x @ W1[e] + b1[e])      # [16384, 2048]
  out = h @ W2[e] + b2[e]            # [16384, 512]

x is transposed host-side (numpy) so the kernel receives xT [D, C] and
needs NO on-chip transposes; d (layer-1 contraction) is already on
partitions.

Per-core dataflow:
  1. DMA xT block [d=512, t=512] -> SBUF [128p, kd, t] (prefetch ahead)
  2. L1: psum[f,t] = sum_k matmul(lhsT=W1[dk, f], rhs=xT[dk, t])   (fp32r)
  3. ScalarE Gelu(+b1 per-partition bias) psum -> hT sbuf [f, t]
  4. L2: psum[t,d'] = sum_k matmul(lhsT=hT[fk, t], rhs=W2[fk, d']) (fp32r)
     -> output lands in natural token-major layout
  5. DVE add b2 (broadcast) psum -> sbuf, DMA out.
"""

import os
import numpy as np

E, C, D, F = 8, 16384, 512, 2048
P = 128
TBLK = 512  # tokens per block
MM_DT = "bfloat16"  # moving operand may stream 2 cols/cyc on HW (vs fp32r 1)

_CACHE = {}


def _build(T, act="Gelu_apprx_tanh"):
    import concourse.mybir as mybir
    import concourse.tile as tile
    from concourse import bacc

    f32 = mybir.dt.float32
    mm_dt = getattr(mybir.dt, MM_DT)
    gelu_fn = getattr(mybir.ActivationFunctionType, act)

    nc = bacc.Bacc("TRN2", target_bir_lowering=False, debug=False)

    xt_d = nc.dram_tensor("xt", [D, T], mm_dt, kind="ExternalInput").ap()
    w1_d = nc.dram_tensor("w1", [D, F], mm_dt, kind="ExternalInput").ap()
    b1_d = nc.dram_tensor("b1", [F], f32, kind="ExternalInput").ap()
    w2_d = nc.dram_tensor("w2", [F, D], mm_dt, kind="ExternalInput").ap()
    b2_d = nc.dram_tensor("b2", [D], f32, kind="ExternalInput").ap()
    o_d = nc.dram_tensor("out", [T, D], f32, kind="ExternalOutput").ap()

    KD = D // P   # 4  k-tiles (d) for layer 1
    KF = F // P   # 16 k-tiles (f) for layer 2
    NB = T // TBLK
    JT = TBLK // P  # 4 token sub-tiles per block

    xt_r = xt_d.rearrange("(k p) t -> p k t", p=P)

    with tile.TileContext(nc) as tc:
        with (
            tc.tile_pool(name="consts", bufs=1) as consts,
            tc.tile_pool(name="xt", bufs=3) as xt_pool,
            tc.tile_pool(name="ht", bufs=2) as ht_pool,
            tc.tile_pool(name="ot", bufs=3) as ot_pool,
            tc.tile_pool(name="ph", bufs=3, space="PSUM") as ph_pool,
            tc.tile_pool(name="po", bufs=4, space="PSUM") as po_pool,
            tc.tile_pool(name="warm", bufs=1, space="PSUM") as warm_pool,
        ):
            def load_block(blk):
                xt = xt_pool.tile([P, KD, TBLK], mm_dt, name="xt", tag="xt")
                t0 = blk * TBLK
                for k in range(KD):
                    nc.sync.dma_start(xt[:, k, :], xt_r[:, k, t0 : t0 + TBLK])
                return xt

            # --- setup ---
            # PE warmup: dummy matmuls on a zeroed scratch tile run during the
            # initial DMA wait so the HAM clock-gate opens (1.2 -> 2.4 GHz)
            # before the first real matmul.
            scratch = consts.tile([P, TBLK], mm_dt)
            nc.gpsimd.memset(scratch[:], 0)
            ph_w = warm_pool.tile([P, TBLK], f32)
            for _ in range(11):
                nc.tensor.matmul(
                    ph_w[:], scratch[:, :P], scratch[:], start=True, stop=True
                )

            # DMA order tracks the startup critical path: biases first (gelu
            # f0 gates ph psum recycling), then xt0/w1 interleaved in the
            # order layer 1 consumes them.
            b1_sb = consts.tile([P, KF], f32)
            nc.sync.dma_start(b1_sb[:], b1_d.rearrange("(k p) -> p k", p=P))
            b2_bc = consts.tile([P, D], f32)
            nc.sync.dma_start(b2_bc[:], b2_d.unsqueeze(0).partition_broadcast(P))

            w1_sb = consts.tile([P, KD, F], mm_dt)
            w1_r = w1_d.rearrange("(k p) f -> p k f", p=P)
            FB = F // 4  # 512-wide f chunks, consumption order
            xts = {}
            xt0 = xt_pool.tile([P, KD, TBLK], mm_dt, name="xt", tag="xt")
            for k in range(KD):
                nc.sync.dma_start(xt0[:, k, :], xt_r[:, k, 0:TBLK])
                nc.sync.dma_start(
                    w1_sb[:, k, 0:FB], w1_r[:, k, 0:FB]
                )
            xts[0] = xt0
            for fb in range(1, 4):
                for k in range(KD):
                    nc.sync.dma_start(
                        w1_sb[:, k, fb * FB : (fb + 1) * FB],
                        w1_r[:, k, fb * FB : (fb + 1) * FB],
                    )

            # w2 before xt1: L2(block0) consumes w2 well before L1(block1)
            # needs xt1 — the reverse order stalls L2 and re-throttles HAM.
            w2_sb = consts.tile([P, KF, D], mm_dt)
            w2_r = w2_d.rearrange("(k p) d -> p k d", p=P)
            for k in range(KF):
                nc.sync.dma_start(w2_sb[:, k, :], w2_r[:, k, :])

            xts[1] = load_block(1)

            def layer1(xt):
                hts = []
                for f in range(KF):
                    ph = ph_pool.tile([P, TBLK], f32)
                    for k in range(KD):
                        nc.tensor.matmul(
                            ph[:],
                            w1_sb[:, k, f * P : (f + 1) * P],
                            xt[:, k, :],
                            start=(k == 0),
                            stop=(k == KD - 1),
                        )
                    ht_f = ht_pool.tile(
                        [P, TBLK], mm_dt, name=f"ht{f}", tag=f"ht{f}"
                    )
                    nc.scalar.activation(
                        ht_f[:], ph[:], gelu_fn, bias=b1_sb[:, f : f + 1]
                    )
                    hts.append(ht_f)
                return hts

            def layer2(blk, hts):
                t0 = blk * TBLK
                for j in range(JT):
                    po = po_pool.tile([P, D], f32)
                    for k in range(KF):
                        nc.tensor.matmul(
                            po[:],
                            hts[k][:, j * P : (j + 1) * P],
                            w2_sb[:, k, :],
                            start=(k == 0),
                            stop=(k == KF - 1),
                        )
                    ot_j = ot_pool.tile([P, D], f32, name=f"ot{j}", tag=f"ot{j}")
                    nc.vector.tensor_add(ot_j[:], po[:], b2_bc[:])
                    nc.sync.dma_start(
                        o_d[t0 + j * P : t0 + (j + 1) * P, :], ot_j[:]
                    )

            for blk in range(NB):
                if blk + 2 < NB:
                    xts[blk + 2] = load_block(blk + 2)
                hts = layer1(xts.pop(blk))
                layer2(blk, hts)

    nc.compile()
    return nc


def _get_nc(T):
    if T not in _CACHE:
        _CACHE[T] = _build(T)
    return _CACHE[T]


def kernel(inputs, W1, b1, W2, b2):
    import ml_dtypes
    from concourse.bass_utils import run_bass_kernel_spmd

    bf16 = ml_dtypes.bfloat16
    inputs = np.asarray(inputs, dtype=np.float32)
    W1 = np.ascontiguousarray(np.asarray(W1, dtype=np.float32).astype(bf16))
    b1 = np.ascontiguousarray(np.asarray(b1, dtype=np.float32))
    W2 = np.ascontiguousarray(np.asarray(W2, dtype=np.float32).astype(bf16))
    b2 = np.ascontiguousarray(np.asarray(b2, dtype=np.float32))
    # host-side transpose: feed xT [D, C] per expert (d on partitions)
    xT = np.ascontiguousarray(inputs[0].transpose(0, 2, 1).astype(bf16))

    nc = _get_nc(C)
    in_maps = [
        {
            "xt": xT[e],
            "w1": W1[e],
            "b1": b1[e],
            "w2": W2[e],
            "b2": b2[e],
        }
        for e in range(E)
    ]
    trace = os.environ.get("KERNEL_TRACE", "0") == "1"
    res = run_bass_kernel_spmd(
        nc, in_maps, core_ids=list(range(E)), trace=trace
    )
    if trace:
        kernel.last_exec_time_ns = res.exec_time_ns
    out = np.stack([res.results[e]["out"] for e in range(E)], axis=0)[None]
    return out
